# revision 34
# baseline (speedup 1.0000x reference)
"""Trainium2 Bass kernel for multi-head attention (B=2, S=2048, D=1024, H=16).

Sharding: data-parallel over query rows with sequence-sharded K/V projection.
Core c handles batch b=c//4 and query rows [512*(c%4), 512*(c%4+1)). Each
core computes K and V projections (all 16 heads) only for its OWN 512-row
sequence block; the blocks are exchanged with the 3 other cores of the same
batch via one AllGather over replica groups [[0..3],[4..7]] (split into two
collectives so wave-0 attention can start while the rest of V is in flight).
This removes the 4x duplicated K/V projection work of the pure
data-parallel scheme. Q projection, attention, and the output projection
stay local to the core's query block, so no reduction collective is needed.

Because each core's local sequence block IS its group rank, the AllGather
output arrives in global sequence order on every core — the SPMD program is
identical across cores; only host-prepped data differs (x^T rolled so the
local block sits at columns 0:512).

Layouts:
  xT   [8,128,2048] bf16  x[b] transposed (d on partitions), s-axis rolled
  kT   per 2-head group [128, 2048] bf16 in GLOBAL s order (from AllGather)
  va   per chunk-pair [128, 2, 4 heads, 65] bf16: v + denominator column
  scores^T [s, q]; softmax denominator comes from the 65th v column
  (matmul output row 64).

Padding mask: V rows (and the denominator column) are multiplied by the 0/1
key mask, so masked keys contribute exactly 0 to the numerator and the
denominator — identical to the reference's -1e9 score masking. Softmax skips
max-subtraction (scores ~N(0,1) after the 1/8 scale; fp32 exp cannot
overflow). bk is dropped entirely: adding bk to K shifts every score of a
query row by the same q.bk constant, which softmax cancels.

All matmuls run in bf16 (inputs pre-rounded on host) with f32 PSUM
accumulation. PSUM->SBUF copies run on DVE; mask multiplies and the
normalize broadcast on the Pool engine; exp on ACT. Weight DMAs are issued
ahead of use; wo is resident early.
"""

import os
import sys

sys.path.insert(0, "/opt/trn_rl_repo")

import numpy as np

B, S, D, H, DH = 2, 2048, 1024, 16, 64
NCORES = 8
CPB = NCORES // B       # cores per batch
QB = S // CPB           # 512 query rows per core
P = 128
DCH = D // P            # 8 contraction chunks
SC = S // P             # 16 s-chunks
NW = 4                  # waves
HPW = H // NW           # heads per wave
NG = H // 2             # 8 head-pair groups
LSC = 4                 # local s-chunks per core

KCOLS = NG * QB          # 4096 k columns in the K gather
VWCOLS = LSC * HPW * DH  # 1024 v columns per wave
AGVC = NW * VWCOLS       # 4096 v columns in the V gather

_compiled = {}
LAST_RESULTS = None
ABLATE = set()
UNROLL = 1
SIMCC = bool(os.environ.get("BASS_SIMCC"))  # sim-only: collectives -> copies


def _build_program():
    import concourse.bass as bass
    import concourse.mybir as mybir
    import concourse.tile as tile
    from concourse import bacc

    f32 = mybir.dt.float32
    bf16 = mybir.dt.bfloat16
    AF = mybir.ActivationFunctionType
    OP = mybir.AluOpType
    RG = [[0, 1, 2, 3], [4, 5, 6, 7]]

    nc = bacc.Bacc(
        "TRN2", target_bir_lowering=False, debug=False,
        num_devices=NCORES,
    )

    xT = nc.dram_tensor("xT", [DCH, P, QB], bf16, kind="ExternalInput")
    wq = nc.dram_tensor("wq", [NG, P, DCH, P], bf16, kind="ExternalInput")
    wk = nc.dram_tensor("wk", [NG, P, DCH, P], bf16, kind="ExternalInput")
    wv = nc.dram_tensor("wv", [NW, P, DCH, 256], bf16, kind="ExternalInput")
    woT = nc.dram_tensor("woT", [DCH, P, D], bf16, kind="ExternalInput")
    bq = nc.dram_tensor("bq", [P, NG], f32, kind="ExternalInput")
    bv = nc.dram_tensor("bv", [1, D], f32, kind="ExternalInput")
    bo = nc.dram_tensor("bo", [1, D], f32, kind="ExternalInput")
    maskT = nc.dram_tensor("maskT", [P, SC], f32, kind="ExternalInput")
    maskTl = nc.dram_tensor("maskTl", [P, LSC], f32, kind="ExternalInput")
    out = nc.dram_tensor("out", [QB, D], f32, kind="ExternalOutput")

    with tile.TileContext(nc) as tc:
        with (
            tc.tile_pool(name="const", bufs=1) as constp,
            tc.tile_pool(name="xt", bufs=DCH) as xtpool,
            tc.tile_pool(name="wo", bufs=DCH) as wopool,
            tc.tile_pool(name="wk", bufs=4) as wkpool,
            tc.tile_pool(name="wq", bufs=NG) as wqpool,
            tc.tile_pool(name="wv", bufs=NW) as wvpool,
            tc.tile_pool(name="kvi", bufs=2) as kvipool,
            tc.tile_pool(name="kt", bufs=4) as ktpool,
            tc.tile_pool(name="va", bufs=1) as vapool,
            tc.tile_pool(name="pt", bufs=8) as ptpool,
            tc.tile_pool(name="cat", bufs=1) as catp,
            tc.tile_pool(name="rr", bufs=2) as rpool,
            tc.tile_pool(name="osb", bufs=2) as outp,
            tc.tile_pool(name="dram", bufs=1, space="DRAM") as dramp,
            tc.tile_pool(name="psc", bufs=3, space="PSUM") as psc,
            tc.tile_pool(name="po", bufs=2, space="PSUM") as pop,
        ):
            for rep in range(UNROLL):
                # ---- weight + x DMAs, interleaved so the PE starts early
                wk_t, xt = [], []
                for g in range(NG):
                    t = wkpool.tile([P, DCH, P], bf16, tag="wk",
                                    name=f"wk{rep}_{g}")
                    if g == 0:
                        nc.sync.dma_start(out=t[:, 0:DCH // 2],
                                          in_=wk[g][:, 0:DCH // 2])
                        nc.sync.dma_start(out=t[:, DCH // 2:DCH],
                                          in_=wk[g][:, DCH // 2:DCH])
                    else:
                        nc.sync.dma_start(out=t[:], in_=wk[g])
                    wk_t.append(t)
                    if g < DCH:
                        t2 = xtpool.tile([P, QB], bf16, tag="xt",
                                         name=f"xt{rep}_{g}")
                        nc.sync.dma_start(out=t2[:], in_=xT[g])
                        xt.append(t2)
                wv_t = []
                for w in range(NW):
                    t = wvpool.tile([P, DCH, 256], bf16, tag="wv",
                                    name=f"wv{rep}_{w}")
                    nc.sync.dma_start(out=t[:], in_=wv[w])
                    wv_t.append(t)

                # ---- small constants
                if rep == 0:
                    bq_sb = constp.tile([P, NG], f32, tag="bq")
                    nc.sync.dma_start(out=bq_sb[:], in_=bq[:])
                    mask_sb = constp.tile([P, SC], f32, tag="mask")
                    nc.sync.dma_start(out=mask_sb[:], in_=maskT[:])
                    maskl_sb = constp.tile([P, LSC], f32, tag="maskl")
                    nc.sync.dma_start(out=maskl_sb[:], in_=maskTl[:])
                    bv_src = constp.tile([1, D], f32, tag="bvs")
                    nc.sync.dma_start(out=bv_src[:], in_=bv[:])
                    bo_src = constp.tile([1, D], f32, tag="bos")
                    nc.sync.dma_start(out=bo_src[:], in_=bo[:])
                    ones_t = constp.tile([1, P], bf16, tag="ones")
                    nc.vector.memset(ones_t[:], 1.0)
                    bv_rep = constp.tile([P, D], f32, tag="bvr")
                    nc.gpsimd.partition_broadcast(bv_rep[:], bv_src[:])
                    bo_rep = constp.tile([P, D], f32, tag="bor")
                    nc.gpsimd.partition_broadcast(bo_rep[:], bo_src[:])
                    # persistent q tiles: dead half zeroed once, live half
                    # rewritten per group; keeps the scores matmul at K=128
                    qz = []
                    for i in range(2 * NG):
                        t = constp.tile([P, QB], bf16, tag=f"qz{i}",
                                        name=f"qz_{i}")
                        par = i % 2
                        olo, ohi = (1 - par) * 64, (2 - par) * 64
                        nc.vector.memset(t[olo:ohi, :], 0.0)
                        qz.append(t)
                    # persistent va pair tiles; denominator column (64) is
                    # written from the global mask on wave 0 only (its
                    # content is identical every wave)
                    m4 = mask_sb[:, 0:HPW].rearrange("p (h e) -> p h e", e=1)

                    def make_vap(ws, pc):
                        t = vapool.tile([P, 2, HPW, 65], bf16,
                                        tag=f"vap{ws}_{pc}",
                                        name=f"vap_{ws}_{pc}")
                        for jj in range(2):
                            # denominator column = global mask; written
                            # once, before the gathers, so the Pool queue
                            # can't stall attention on it
                            nc.gpsimd.tensor_scalar(
                                t[:, jj, :, 64:65], m4, 0.0,
                                mask_sb[:, 2 * pc + jj:2 * pc + jj + 1],
                                OP.mult, OP.add,
                            )
                        return t

                    vap2 = [[make_vap(ws, pc) for pc in range(SC // 2)]
                            for ws in range(3)]

                wq_t = []
                for g in range(NG):
                    t = wqpool.tile([P, DCH, P], bf16, tag="wq",
                                    name=f"wq{rep}_{g}")
                    nc.sync.dma_start(out=t[:], in_=wq[g])
                    wq_t.append(t)

                # ---- wo resident early (own pool: no WAR on xt)
                wo_sb = []
                for c in range(DCH):
                    t = wopool.tile([P, D], bf16, tag="wo", name=f"wo{rep}_{c}")
                    nc.sync.dma_start(out=t[:], in_=woT[c])
                    wo_sb.append(t)

                concat = catp.tile([P, DCH, QB], bf16, tag="cat",
                                   name=f"cat{rep}")

                # ---- local K projection: all 16 heads for this core's own
                # 512-row block. Projection psums borrow the (idle) psc
                # banks so 4 are in flight and the DVE drain never stalls
                # the PE.
                kv1 = kvipool.tile([P, KCOLS], bf16, tag="kv1",
                                   name=f"kv1_{rep}")
                DRC = 1 if "kv1" in ABLATE else DCH

                def proj_psum(name):
                    return psc.tile([P, 2, QB], f32, tag="ps", name=name)

                def k_local(g, ps):
                    for d in range(DRC):
                        nc.tensor.matmul(
                            ps,
                            wk_t[g][:, d, :],
                            xt[d][:, 0:QB],
                            start=(d == 0),
                            stop=(d == DRC - 1),
                        )
                    nc.vector.tensor_copy(kv1[:, g * QB:(g + 1) * QB], ps)

                # Per-wave K gathers: each wave needs only its own 2 head
                # groups, so its gather launches the moment they project.
                KW = 2 * QB
                db_k = [dramp.tile([P, KW], bf16, tag=f"dbik{h}",
                                   name=f"dbik{h}_{rep}") for h in range(NW)]
                db_ok = [dramp.tile([CPB, P, KW], bf16, tag=f"dbok{h}",
                                    name=f"dbok{h}_{rep}") for h in range(NW)]

                def k_half(w):
                    pst = proj_psum(f"kps_{rep}_{w}")
                    k_local(2 * w, pst[:, 0, :])
                    k_local(2 * w + 1, pst[:, 1, :])
                    nc.gpsimd.dma_start(
                        db_k[w][:], kv1[:, 2 * w * QB:(2 * w + 2) * QB])
                    if SIMCC:
                        for j in range(CPB):
                            nc.gpsimd.dma_start(db_ok[w][j], db_k[w][:])
                    else:
                        nc.gpsimd.collective_compute(
                            "AllGather", mybir.AluOpType.bypass,
                            replica_groups=RG,
                            ins=[db_k[w].opt()],
                            outs=[db_ok[w].opt()],
                        )

                def v_local(w, dst, base):
                    # local V for wave w (4 heads), 4 local s-chunks
                    for sc in range(LSC):
                        if sc % 2 == 0:
                            vpst = proj_psum(f"vps_{rep}_{w}_{sc // 2}")
                        ps = vpst[:, sc % 2, 0:256]
                        for d in range(DRC):
                            nc.tensor.matmul(
                                ps[:],
                                xt[d][:, sc * P:(sc + 1) * P],
                                wv_t[w][:, d, :],
                                start=(d == 0),
                                stop=(d == DRC - 1),
                            )
                        o = dst[:, base + sc * 256:base + (sc + 1) * 256]
                        o_r = o.rearrange("p (h e) -> p h e", e=DH)
                        ps_r = ps.rearrange("p (h e) -> p h e", e=DH)
                        nc.vector.tensor_tensor(
                            o_r, ps_r,
                            bv_rep[:, w * 256:(w + 1) * 256].rearrange(
                                "p (h e) -> p h e", e=DH),
                            OP.add,
                        )
                        # zero masked local key rows (Pool, local mask col)
                        nc.gpsimd.tensor_scalar(
                            o_r, o_r, maskl_sb[:, sc:sc + 1], None, OP.mult,
                        )

                kv2 = kvipool.tile([P, AGVC], bf16, tag="kv2",
                                   name=f"kv2_{rep}")
                db_v = [dramp.tile([P, VWCOLS], bf16, tag=f"dbiv{h}",
                                   name=f"dbiv{h}_{rep}") for h in range(NW)]
                db_ov = [dramp.tile([CPB, P, VWCOLS], bf16, tag=f"dbov{h}",
                                    name=f"dbov{h}_{rep}") for h in range(NW)]

                def v_wave(w):
                    v_local(w, kv2, w * VWCOLS)
                    nc.gpsimd.dma_start(
                        db_v[w][:], kv2[:, w * VWCOLS:(w + 1) * VWCOLS])
                    if SIMCC:
                        for j in range(CPB):
                            nc.gpsimd.dma_start(db_ov[w][j], db_v[w][:])
                    else:
                        nc.gpsimd.collective_compute(
                            "AllGather", mybir.AluOpType.bypass,
                            replica_groups=RG,
                            ins=[db_v[w].opt()],
                            outs=[db_ov[w].opt()],
                        )

                # interleave so each wave's K and V gather as early as
                # possible, in the order the waves consume them
                k_half(0)
                v_wave(0)
                k_half(1)
                v_wave(1)
                k_half(2)
                v_wave(2)
                k_half(3)
                v_wave(3)

                # ---- Q projection for all 8 groups (overlaps the gathers)
                for g in range(NG):
                    if g % 2 == 0:
                        qpst = proj_psum(f"qps_{rep}_{g // 2}")
                    ps = qpst[:, g % 2, :]
                    for d in range(DCH):
                        nc.tensor.matmul(
                            ps,
                            wq_t[g][:, d, :],
                            xt[d][:, 0:QB],
                            start=(d == 0),
                            stop=(d == DCH - 1),
                        )
                    for par in range(2):
                        lo, hi = par * 64, (par + 1) * 64
                        nc.vector.tensor_scalar_add(
                            qz[2 * g + par][lo:hi, :], ps[lo:hi, :],
                            bq_sb[lo:hi, g:g + 1],
                        )

                def assemble_wave(wave):
                    vap = vap2[wave % 3]
                    groups = [2 * wave, 2 * wave + 1]
                    kt = []
                    for gl, g in enumerate(groups):
                        ktile = ktpool.tile([P, S], bf16, tag="kt",
                                            name=f"kt_{rep}_{wave}_{gl}")
                        dbo, gc = db_ok[g // 2], g % 2
                        for j in range(CPB):
                            nc.sync.dma_start(
                                out=ktile[:, j * QB:(j + 1) * QB],
                                in_=dbo[j][:, gc * QB:(gc + 1) * QB],
                            )
                        kt.append(ktile)
                    for sc in range(SC):
                        j, k = sc // LSC, sc % LSC
                        src = db_ov[wave][j][:, k * 256:(k + 1) * 256]
                        nc.sync.dma_start(
                            out=vap[sc // 2][:, sc % 2, :, 0:DH],
                            in_=src.rearrange("p (h e) -> p h e", e=DH),
                        )
                    return kt

                def attend_head(wave, kt, hl):
                    vap = vap2[wave % 3]
                    gl, par = hl // 2, hl % 2
                    po_t = pop.tile([P, QB], f32, tag="po")
                    pts = {}

                    def emit_scores_pair(pc):
                        # two s-chunks -> one [128, 1024] psum (2 banks),
                        # one exp over both
                        sps = psc.tile([P, 2, QB], f32, tag="ps")
                        for j in range(2):
                            sc = 2 * pc + j
                            nc.tensor.matmul(
                                sps[:, j, :],
                                kt[gl][:, sc * P:(sc + 1) * P],
                                qz[2 * (2 * wave + gl) + par][:],
                                start=True,
                                stop=True,
                            )
                        pt = ptpool.tile([P, 2, QB], bf16, tag="pt")
                        nc.scalar.activation(
                            pt[:], sps[:], AF.Exp,
                            bias=0.0, scale=0.125,
                        )
                        pts[pc] = pt

                    def emit_o(pc):
                        pt = pts.pop(pc)
                        for j in range(2):
                            sc = 2 * pc + j
                            nc.tensor.matmul(
                                po_t[0:65, :],
                                vap[pc][:, j, hl, :],
                                pt[:, j, :],
                                start=(sc == 0),
                                stop=(sc == SC - 1),
                            )

                    NP = SC // 2
                    emit_scores_pair(0)
                    emit_scores_pair(1)
                    # prev head's tail runs after this head's first scores,
                    # so the ACT queue is never empty at a head boundary
                    if attend_head.pending is not None:
                        attend_head.pending()
                        attend_head.pending = None
                    for pc in range(2, NP):
                        emit_o(pc - 2)
                        emit_scores_pair(pc)

                    def finish():
                        emit_o(NP - 2)
                        emit_o(NP - 1)
                        normalize()
                    attend_head.pending = finish

                    def normalize():
                        finish_normalize(wave, hl, po_t)

                def finish_normalize(wave, hl, po_t):
                    gl, par = hl // 2, hl % 2
                    final = wave == NW - 1 and hl == 2
                    # normalize: row 64 of po_t is the softmax denominator.
                    # Copy PSUM->SBUF first: frees the po bank immediately,
                    # and the rest of the chain is SBUF-only (2-byte ops).
                    pod = rpool.tile([65, QB], bf16, tag="pod")
                    with nc.allow_low_precision(reason="softmax bf16"):
                        nc.vector.tensor_copy(pod[:], po_t[0:65, :])
                        nc.vector.reciprocal(pod[64:65, :], pod[64:65, :])
                    # partition_broadcast requires a base-0 input on HW
                    den0 = rpool.tile([1, QB], bf16, tag="den0")
                    nc.sync.dma_start(out=den0[:], in_=pod[64:65, :])
                    if final:
                        # very last head: broadcast on the PE (in-order,
                        # instant) so the o-proj isn't stalled behind the
                        # Pool queue; earlier heads' tails are already
                        # hidden by the cross-head pipeline
                        rep_ps = psc.tile([P, 2, QB], f32, tag="ps",
                                          name=f"repps_{rep}_{wave}_{hl}")
                        nc.tensor.matmul(rep_ps[:, 0, :], ones_t[0:1, :],
                                         den0[0:1, :], start=True, stop=True)
                        rep_ap = rep_ps[:, 0, :]
                    else:
                        rep_t = rpool.tile([P, QB], bf16, tag="rep")
                        nc.gpsimd.partition_broadcast(rep_t[:], den0[0:1, :])
                        rep_ap = rep_t[:]
                    cslot = wave * 2 + gl
                    if par == 0:
                        nc.vector.tensor_tensor(
                            concat[0:64, cslot, :], pod[0:64, :],
                            rep_ap[0:64, :], OP.mult,
                        )
                    else:
                        tmp = rpool.tile([64, QB], bf16, tag="tmp")
                        nc.vector.tensor_tensor(
                            tmp[:], pod[0:64, :], rep_ap[0:64, :], OP.mult,
                        )
                        nc.sync.dma_start(
                            out=concat[64:P, cslot, :], in_=tmp[:],
                        )

                attend_head.pending = None
                kt = assemble_wave(0)
                for wave in range(NW):
                    kt_next = (assemble_wave(wave + 1)
                               if wave + 1 < NW else None)
                    # par-1 heads first: their concat writes go through a
                    # DMA, so give them the longest time to drain
                    for hl in (1, 3, 0, 2):
                        attend_head(wave, kt, hl)
                    kt = kt_next
                attend_head.pending()
                attend_head.pending = None

                # ---- output projection (contraction over h*dh chunks)
                def emit_oproj(qt_i, eb, split_tail=False):
                    if eb == 0:
                        emit_oproj.pst = proj_psum(f"ops_{rep}_{qt_i}")
                    ps = emit_oproj.pst[:, eb, :]
                    CR = 1 if "c1" in ABLATE else DCH
                    for c in range(CR):
                        nc.tensor.matmul(
                            ps,
                            concat[:, c, qt_i * P:(qt_i + 1) * P],
                            wo_sb[c][:, eb * 512:(eb + 1) * 512],
                            start=(c == 0),
                            stop=(c == CR - 1),
                        )
                    osb = outp.tile([P, 512], f32, tag="osb")
                    halves = (slice(0, 256), slice(256, 512)) if split_tail \
                        else (slice(0, 512),)
                    for hs in halves:
                        nc.vector.tensor_tensor(
                            osb[:, hs], ps[:, hs],
                            bo_rep[:, eb * 512 + hs.start:
                                   eb * 512 + hs.stop],
                            OP.add,
                        )
                        nc.sync.dma_start(
                            out=out[qt_i * P:(qt_i + 1) * P,
                                    eb * 512 + hs.start:eb * 512 + hs.stop],
                            in_=osb[:, hs],
                        )

                for qt_i in range(QB // P):
                    for eb in range(2):
                        if qt_i == QB // P - 1 and eb == 1:
                            emit_oproj(qt_i, eb, split_tail=True)
                        else:
                            emit_oproj(qt_i, eb)

    nc.compile()
    nc.finalize()
    return nc


def _to_bf16(a):
    import ml_dtypes
    return np.asarray(a, dtype=np.float32).astype(ml_dtypes.bfloat16)


def prep_inputs(x, pad_mask, wq, wk, wv, bq, bk, bv, wo, bo):
    """Build per-core input maps (host-side shard + layout prep)."""
    x = np.ascontiguousarray(np.asarray(x, dtype=np.float32))
    pad_mask = np.asarray(pad_mask)
    wq = np.asarray(wq, dtype=np.float32)
    wk = np.asarray(wk, dtype=np.float32)
    wv = np.asarray(wv, dtype=np.float32)
    bq = np.asarray(bq, dtype=np.float32)
    bv = np.asarray(bv, dtype=np.float32)
    wo = np.asarray(wo, dtype=np.float32)
    bo = np.asarray(bo, dtype=np.float32)

    # weights: [H, D, DH] -> [d, h*dh] (h-major columns)
    def stack_groups(w, gsz):
        ws = np.ascontiguousarray(w.transpose(1, 0, 2).reshape(D, D))
        m = gsz * DH
        arr = ws.reshape(DCH, P, H // gsz, m).transpose(2, 1, 0, 3)
        return np.ascontiguousarray(arr)

    wq_dev = _to_bf16(stack_groups(wq, 2))
    wk_dev = _to_bf16(stack_groups(wk, 2))
    wv_dev = _to_bf16(stack_groups(wv, 4))
    woT_dev = _to_bf16(np.ascontiguousarray(wo.T).reshape(DCH, P, D))
    bq_dev = np.ascontiguousarray(bq.reshape(NG, P).T)
    bv_dev = np.ascontiguousarray(bv.reshape(1, D))
    bo_dev = np.ascontiguousarray(bo.reshape(1, D))

    in_maps = []
    for c in range(NCORES):
        b, qo = c // CPB, c % CPB
        # only this core's own 512-row block of x is ever read on-device
        xt = x[b][qo * QB:(qo + 1) * QB, :].T  # [D, QB]
        xt_dev = _to_bf16(np.ascontiguousarray(xt)).reshape(DCH, P, QB)
        m01 = (pad_mask[b] != 0).astype(np.float32)          # global order
        maskT_dev = np.ascontiguousarray(m01.reshape(SC, P).T)
        ml = m01[qo * QB:(qo + 1) * QB]                      # local block
        maskTl_dev = np.ascontiguousarray(ml.reshape(LSC, P).T)
        in_maps.append({
            "xT": xt_dev, "wq": wq_dev, "wk": wk_dev, "wv": wv_dev,
            "woT": woT_dev, "bq": bq_dev, "bv": bv_dev,
            "bo": bo_dev, "maskT": maskT_dev, "maskTl": maskTl_dev,
        })
    return in_maps


def kernel(**inputs):
    global LAST_RESULTS
    from concourse.bass_utils import run_bass_kernel_spmd

    if "nc" not in _compiled:
        _compiled["nc"] = _build_program()
    nc = _compiled["nc"]

    in_maps = prep_inputs(**inputs)
    res = run_bass_kernel_spmd(
        nc, in_maps, list(range(NCORES)),
        trace=bool(os.environ.get("BASS_TRACE")),
    )
    LAST_RESULTS = res

    out = np.empty((B, S, D), dtype=np.float32)
    for c in range(NCORES):
        b, qo = c // CPB, c % CPB
        out[b, qo * QB:(qo + 1) * QB, :] = res.results[c]["out"]
    return out
